# revision 40
# baseline (speedup 1.0000x reference)
"""MLA causal self-attention (shared latent K=V) on 8 Trainium2 NeuronCores.

Reference computation (B=2, T=2048, C=2048, NH=16, L=128):
    k   = x @ W_lat + b_lat                      [B,T,L]   shared latent (K and V)
    q   = (x @ W_d + b_d) -> [B,NH,T,L]
    att = softmax(causal(q @ k^T / sqrt(L)))     [B,NH,T,T]
    y   = att @ k -> [B,T,NH*L]
    out = y @ W_proj + b_proj                    [B,T,C]

Sharding: core = 4*b + g handles batch b, head group g (4 heads).  The latent
k is computed per-core (replicated), so there is no cross-device traffic.
Each core returns partial out = y_g @ W_proj[rows of g]; host sums the 4
group partials per batch and adds b_proj.

Device kernel layout notes:
  - All matmuls in bf16 (inputs cast on host), f32 PSUM accumulation.
  - Scores are computed transposed, ST[s,t] = k_s . q_t (per head), so
    exp(ST) is directly P^T, which is the stationary operand needed by the
    second matmul y[t,l] = sum_s P^T[s,t] * k_aug[s,l].  k_aug carries an
    extra ones column, making column L of y the softmax denominator Z[t].
  - Softmax skips max-subtraction: scores are O(+-5) here, exp is safe in f32.
  - Causal masking: off-diagonal [128s x 512t] blocks are fully valid or
    fully skipped; the 4 diagonal block flavours are masked by a
    precomputed 0/1 mask multiply after exp.
"""
import contextlib
import math
import sys

sys.path.insert(0, "/opt/trn_rl_repo")

import numpy as np
import ml_dtypes

import concourse.bass as bass
import concourse.mybir as mybir
import concourse.tile as tile
from concourse import bass_utils
from concourse.bass import ts
from concourse.vector_clock import ScopedClock

BF16 = ml_dtypes.bfloat16
F32 = mybir.dt.float32
F16 = mybir.dt.float16
BF = mybir.dt.bfloat16

B, T, C = 2, 2048, 2048
NH, L = 16, 128
HPC = 4          # heads per core
G = NH // HPC    # head groups (= cores per batch)
P = 128
CO = C // P      # c-tiles (contraction for q/k projections)
TT = T // P      # 128-row tiles of T
TB = T // 512    # 512-col blocks of T
SCALE = 1.0 / math.sqrt(L)

_PATCHED = False


def _patch_tile_drain():
    """This walrus build rejects instructions carrying more than one sync
    wait ("Too many sync wait commands", setupSyncWait).  Two patches:
    (1) a pass over the scheduled instruction lists that hoists all but one
    wait per instruction onto preceding same-engine NoOps, and (2) the
    TileContext tail drain emits one wait_ge instruction per semaphore
    instead of stacking waits on the Drain."""
    global _PATCHED
    if _PATCHED:
        return
    _PATCHED = True

    _orig_lower = tile.TileContext._lower_ordered_insts
    _ctr = [0]

    def _lower_ordered_insts(self, ordered):
        for bbname, insts in ordered.items():
            needs = any(
                i.sync_info is not None and len(i.sync_info.on_wait) > 1
                for i in insts
            )
            if not needs:
                continue
            newlist = []
            for inst in insts:
                si = inst.sync_info
                if si is not None and len(si.on_wait) > 1:
                    waits = list(si.on_wait)
                    for w in waits[:-1]:
                        nop = mybir.InstNoOp(
                            name=f"waitsplit_{_ctr[0]}", ins=[], outs=[],
                            engine=inst.engine,
                        )
                        _ctr[0] += 1
                        nop.sync_info = mybir.SyncInfo(on_wait=[w], on_update=[])
                        newlist.append(nop)
                    inst.sync_info = mybir.SyncInfo(
                        on_wait=[waits[-1]], on_update=list(si.on_update)
                    )
                newlist.append(inst)
            ordered[bbname] = newlist
        return _orig_lower(self, ordered)

    tile.TileContext._lower_ordered_insts = _lower_ordered_insts

    def _drain_and_barrier(self, tick_clock, wait_clock):
        nc = self.nc
        probe = mybir.InstNoOp(
            name="tail_wait_probe", ins=[], outs=[], engine=mybir.EngineType.SP
        )
        wait_clock.add_sem_waits(probe, ScopedClock({None: tick_clock.global_clock}))
        by_num = {h.num: h for h in wait_clock.sems.allocated().values()}
        if probe.sync_info is not None:
            for w in probe.sync_info.on_wait:
                nc.sync.wait_ge(by_num[w.id], w.wait_value)
        nc.sync.drain()
        nc.all_engine_barrier()
        assert self.sems is not None
        popped = nc._tile_sem_poison_stack.pop()
        assert popped is self._sem_poison
        nc.clear_and_free_semaphores(list(self.sems.allocated().values()))
        nc.all_engine_barrier()

    tile.TileContext._drain_and_barrier = _drain_and_barrier


_NC_CACHE = None


def build_nc():
    """Build the single-core Bass program (SPMD across the 8 cores)."""
    global _NC_CACHE
    if _NC_CACHE is not None:
        return _NC_CACHE
    _patch_tile_drain()

    nc = bass.Bass("TRN2", target_bir_lowering=False, debug=False)

    xT = nc.declare_dram_parameter("xT", [C, T], BF, isOutput=False)
    wd = nc.declare_dram_parameter("wd", [C, HPC * L], BF, isOutput=False)
    wlat = nc.declare_dram_parameter("wlat", [C, L], BF, isOutput=False)
    wproj = nc.declare_dram_parameter("wproj", [HPC * L, C], BF, isOutput=False)
    bd = nc.declare_dram_parameter("bd", [P, HPC], F32, isOutput=False)
    blat = nc.declare_dram_parameter("blat", [P, 1], F32, isOutput=False)
    mask = nc.declare_dram_parameter("mask", [P, 4 * 512], BF, isOutput=False)
    ident = nc.declare_dram_parameter("ident", [P, P], BF, isOutput=False)
    out = nc.declare_dram_parameter("out", [T, C], F16, isOutput=True)

    with tile.TileContext(nc) as tc:
        with (
            tc.tile_pool(name="const", bufs=1) as cp,
            tc.tile_pool(name="ptp", bufs=2) as ptp,
            tc.tile_pool(name="small", bufs=3) as sp,
            tc.tile_pool(name="outp", bufs=3) as op_,
        ):
            # ---- constant / persistent SBUF tiles ----
            xT_sb = cp.tile([P, CO, T], BF, tag="xT")
            wd_sb = cp.tile([P, CO, HPC * L], BF, tag="wd")
            wlat_sb = cp.tile([P, CO, L], BF, tag="wlat")
            wproj_sb = cp.tile([P, HPC, C], BF, tag="wproj")
            bd_sb = cp.tile([P, HPC], F32, tag="bd")
            blat_sb = cp.tile([P, 1], F32, tag="blat")
            mask_sb = cp.tile([P, 4 * 512], BF, tag="mask")
            ident_sb = cp.tile([P, P], BF, tag="ident")
            kT_sb = cp.tile([P, T], BF, tag="kT")            # [L, s]
            kaug_sb = cp.tile([P, TT, L + 8], BF, tag="kaug")  # [s, st, l|1]
            qT_sb = cp.tile([P, HPC, T], BF, tag="qT")       # [L, h, t]
            yT_sb = cp.tile([P, HPC, T], BF, tag="yT")       # [L, h, t]

            # Load order matters for the PE ramp: each ramp step co needs
            # wlat[co], wd[co] and xT[co]; xT streams one c-tile per DMA so
            # the ramp is never starved by one large transfer.  mask/wproj
            # only gate later phases.
            nc.scalar.dma_start(
                wlat_sb[:, 0:4, :], wlat[0 : 4 * P, :].rearrange("(o p) l -> p o l", p=P)
            )
            nc.sync.dma_start(
                xT_sb[:, 0:1, :], xT[0:P, :].rearrange("(o p) t -> p o t", p=P)
            )
            nc.scalar.dma_start(
                wd_sb[:, 0:4, :], wd[0 : 4 * P, :].rearrange("(o p) l -> p o l", p=P)
            )
            nc.scalar.dma_start(blat_sb[:], blat[:])
            nc.scalar.dma_start(bd_sb[:], bd[:])
            nc.scalar.dma_start(
                wlat_sb[:, 4:CO, :],
                wlat[4 * P :, :].rearrange("(o p) l -> p o l", p=P),
            )
            nc.scalar.dma_start(
                wd_sb[:, 4:CO, :], wd[4 * P :, :].rearrange("(o p) l -> p o l", p=P)
            )
            nc.scalar.dma_start(ident_sb[:], ident[:])
            xq = [(i, 1) for i in range(1, CO)]
            for off, n in xq:
                nc.sync.dma_start(
                    xT_sb[:, off : off + n, :],
                    xT[off * P : (off + n) * P, :].rearrange("(o p) t -> p o t", p=P),
                )
            nc.scalar.dma_start(mask_sb[:], mask[:])
            for q in range(2):
                nc.scalar.dma_start(
                    wproj_sb[:, 2 * q : 2 * q + 2, :],
                    wproj[ts(q, 2 * P), :].rearrange("(o p) c -> p o c", p=P),
                )

            # ---- phase 1 (ramp): kT and qT[h0] accumulate per c-tile as the
            # xT DMA batches land, keeping PE fed during the 8 MB load.
            # 8 concurrent PSUM accumulation groups in a scoped pool whose
            # banks are released to the later pools.
            with tc.tile_pool(name="ps_ramp", bufs=1, space="PSUM") as ramp:
                kps = [ramp.tile([P, 512], F32, tag=f"kps{tb}", name=f"kps{tb}") for tb in range(TB)]
                qps = [ramp.tile([P, 512], F32, tag=f"qps{tb}", name=f"qps{tb}") for tb in range(TB)]
                for co in range(CO):
                    for tb in range(TB):
                        nc.tensor.matmul(
                            kps[tb][:], wlat_sb[:, co, :], xT_sb[:, co, ts(tb, 512)],
                            start=(co == 0), stop=(co == CO - 1),
                        )
                    for tb in range(TB):
                        nc.tensor.matmul(
                            qps[tb][:], wd_sb[:, co, ts(0, P)], xT_sb[:, co, ts(tb, 512)],
                            start=(co == 0), stop=(co == CO - 1),
                        )
                for tb in range(TB):
                    nc.vector.tensor_scalar_add(
                        kT_sb[:, ts(tb, 512)], kps[tb][:], blat_sb[:, 0:1]
                    )
                    nc.vector.tensor_scalar_add(
                        qT_sb[:, 0, ts(tb, 512)], qps[tb][:], bd_sb[:, 0:1]
                    )

            main_pools = contextlib.ExitStack()
            # PSUM budget (8 banks): psST 2x[128,1024]=4, mm512 2, psy 1, pstr 1
            psST = main_pools.enter_context(
                tc.tile_pool(name="psST", bufs=2, space="PSUM")
            )
            ps512 = main_pools.enter_context(
                tc.tile_pool(name="ps512", bufs=2, space="PSUM")
            )
            psy = main_pools.enter_context(
                tc.tile_pool(name="psy", bufs=1, space="PSUM")
            )
            pstr = main_pools.enter_context(
                tc.tile_pool(name="pstr", bufs=1, space="PSUM")
            )

            # k_aug[s, l|1]: transpose kT tiles via identity matmul, ones col
            nc.vector.memset(kaug_sb[:, :, L : L + 1], 1.0)
            for st in range(TT):
                pt = pstr.tile([P, P], F32, tag="tr")
                nc.tensor.matmul(pt[:], kT_sb[:, ts(st, P)], ident_sb[:], start=True, stop=True)
                nc.vector.tensor_copy(out=kaug_sb[:, st, 0:L], in_=pt[:])

            def qT_phase(h):
                for tb in range(TB):
                    ps = ps512.tile([P, 512], F32, tag="mm512", name=f"q{h}{tb}")
                    for co in range(CO):
                        nc.tensor.matmul(
                            ps[:], wd_sb[:, co, ts(h, P)], xT_sb[:, co, ts(tb, 512)],
                            start=(co == 0), stop=(co == CO - 1),
                        )
                    nc.vector.tensor_scalar_add(
                        qT_sb[:, h, ts(tb, 512)], ps[:], bd_sb[:, h : h + 1]
                    )

            def att_block(J, h):
                """Attention for head h, query block J (512 t's)."""
                ns = 4 * J + 4  # s-tiles covering s <= max t of this block
                ptile = ptp.tile([P, TT, 512], BF, tag="pt", name=f"pt{J}{h}")
                # scores transposed, two s-tiles per PSUM tile.  Off-diagonal
                # pairs share one full-width exp; diagonal s-tiles (st = 4J+r)
                # only have valid t in [128r, 512) -> trim the matmul, exp and
                # mask to that range (the skipped PT region is never read by
                # the y matmuls, which use s-tile st only for t-tiles >= r).
                for sp_i in range(ns // 2):
                    ps = psST.tile([P, 2, 512], F32, tag="st", name=f"st{J}{h}{sp_i}")
                    st0 = 2 * sp_i
                    diag = st0 >= 4 * J
                    if not diag:
                        for k_ in range(2):
                            st = st0 + k_
                            nc.tensor.matmul(
                                ps[:, k_, :], kT_sb[:, ts(st, P)],
                                qT_sb[:, h, ts(J, 512)],
                                start=True, stop=True,
                            )
                        nc.scalar.activation(
                            ptile[:, st0 : st0 + 2, :], ps[:],
                            mybir.ActivationFunctionType.Exp,
                        )
                    else:
                        for k_ in range(2):
                            st = st0 + k_
                            r = st - 4 * J
                            nc.tensor.matmul(
                                ps[:, k_, P * r : 512], kT_sb[:, ts(st, P)],
                                qT_sb[:, h, 512 * J + P * r : 512 * (J + 1)],
                                start=True, stop=True,
                            )
                            nc.scalar.activation(
                                ptile[:, st, P * r : 512], ps[:, k_, P * r : 512],
                                mybir.ActivationFunctionType.Exp,
                            )
                            nc.vector.tensor_tensor(
                                ptile[:, st, P * r : 512],
                                ptile[:, st, P * r : 512],
                                mask_sb[:, 512 * r + P * r : 512 * (r + 1)],
                                mybir.AluOpType.mult,
                            )
                for tt in range(4):
                    tq = 4 * J + tt
                    nsy = tq + 1
                    py = psy.tile([P, 132], F32, tag="yaug", name=f"y{J}{h}{tt}")
                    for st in range(nsy):
                        nc.tensor.matmul(
                            py[:, 0 : L + 1],
                            ptile[:, st, ts(tt, P)],
                            kaug_sb[:, st, 0 : L + 1],
                            start=(st == 0), stop=(st == nsy - 1),
                        )
                    zr = sp.tile([P, 1], F32, tag="zr", name=f"zr{J}{h}{tt}")
                    nc.vector.reciprocal(zr[:], py[:, L : L + 1])
                    ynorm = sp.tile([P, P], BF, tag="ynorm", name=f"yn{J}{h}{tt}")
                    nc.vector.tensor_scalar_mul(ynorm[:], py[:, 0:L], zr[:])
                    ptr = pstr.tile([P, P], F32, tag="tr", name=f"ytr{J}{h}{tt}")
                    nc.tensor.matmul(ptr[:], ynorm[:], ident_sb[:], start=True, stop=True)
                    nc.vector.tensor_copy(out=yT_sb[:, h, ts(tq, P)], in_=ptr[:])

            def proj_rows(J):
                """out rows for t-tiles of block J (needs all heads' yT)."""
                for tq in range(4 * J, 4 * J + 4):
                    osb = op_.tile([P, C], F16, tag="osb", name=f"osb{tq}")
                    for cb in range(4):
                        ps = ps512.tile([P, 512], F32, tag="mm512", name=f"o{tq}{cb}")
                        for h in range(HPC):
                            nc.tensor.matmul(
                                ps[:], yT_sb[:, h, ts(tq, P)],
                                wproj_sb[:, h, ts(cb, 512)],
                                start=(h == 0), stop=(h == HPC - 1),
                            )
                        # all copies on DVE: ACT would pay a ~1.3us table swap
                        # whenever Copy interleaves with attention's Exp
                        nc.vector.tensor_copy(out=osb[:, ts(cb, 512)], in_=ps[:])
                        if tq >= TT - 2:
                            # last rows: drain each block immediately
                            nc.sync.dma_start(
                                out[ts(tq, P), ts(cb, 512)], osb[:, ts(cb, 512)]
                            )
                    if tq < TT - 2:
                        eng = nc.sync if tq % 2 == 0 else nc.scalar
                        eng.dma_start(out[ts(tq, P), :], osb[:])

            # ---- phases 1b + 2 interleaved: attention for (J=0, h) starts as
            # soon as qT[h] exists; qT[h+1] computes under the exp of block h.
            # Projection runs as a tail phase (interleaving it with attention
            # costs ~18us of PSUM-slot contention per the cost model).
            att_block(0, 0)
            for h in range(1, HPC):
                qT_phase(h)
                att_block(0, h)
            for J in range(1, TB):
                for h in range(HPC):
                    att_block(J, h)
            for J in range(TB):
                proj_rows(J)

            main_pools.close()

    _NC_CACHE = nc
    return nc


def make_in_maps(x, W_lat, b_lat, W_d, b_d, W_proj, b_proj):
    """Shard + preprocess full inputs into the 8 per-core input maps."""
    x = np.asarray(x, dtype=np.float32)
    W_lat = np.asarray(W_lat, dtype=np.float32)
    b_lat = np.asarray(b_lat, dtype=np.float32)
    W_d = np.asarray(W_d, dtype=np.float32)
    b_d = np.asarray(b_d, dtype=np.float32)
    W_proj = np.asarray(W_proj, dtype=np.float32)

    # causal mask for transposed-score diagonal blocks:
    # mask[p, r*512 + f] = 1 if f >= 128*r + p else 0
    pp = np.arange(P)[:, None]
    ff = np.arange(512)[None, :]
    mask = np.concatenate(
        [(ff >= P * r + pp) for r in range(4)], axis=1
    ).astype(BF16)
    ident = np.eye(P, dtype=BF16)
    wlat_b = W_lat.astype(BF16)
    blat_b = b_lat.reshape(P, 1).astype(np.float32)

    xT_b = [np.ascontiguousarray(x[b].T).astype(BF16) for b in range(B)]

    in_maps = []
    for core in range(8):
        b, g = core // G, core % G
        sl = slice(512 * g, 512 * (g + 1))
        in_maps.append({
            "xT": xT_b[b],
            "wd": (W_d[:, sl] * SCALE).astype(BF16),
            "wlat": wlat_b,
            "wproj": W_proj[sl, :].astype(BF16),
            "bd": np.ascontiguousarray(
                (b_d[sl] * SCALE).astype(np.float32).reshape(HPC, P).T
            ),
            "blat": blat_b,
            "mask": mask,
            "ident": ident,
        })
    return in_maps


def combine_outputs(per_core_out, b_proj):
    """Sum head-group partials per batch, add b_proj."""
    b_proj = np.asarray(b_proj, dtype=np.float32)
    outs = []
    for b in range(B):
        acc = per_core_out[G * b].astype(np.float32).copy()
        for g in range(1, G):
            acc += per_core_out[G * b + g]
        acc += b_proj[None, :]
        outs.append(acc)
    return np.stack(outs, axis=0)


def kernel(x, W_lat, b_lat, W_d, b_d, W_proj, b_proj):
    nc = build_nc()
    in_maps = make_in_maps(x, W_lat, b_lat, W_d, b_d, W_proj, b_proj)
    res = bass_utils.run_bass_kernel_spmd(nc, in_maps, core_ids=list(range(8)))
    per_core = [res.results[i]["out"] for i in range(8)]
    return combine_outputs(per_core, b_proj)


# revision 42
# speedup vs baseline: 1.5692x; 1.5692x over previous
"""MLA causal self-attention (shared latent K=V) on 8 Trainium2 NeuronCores.

Reference computation (B=2, T=2048, C=2048, NH=16, L=128):
    k   = x @ W_lat + b_lat                      [B,T,L]   shared latent (K and V)
    q   = (x @ W_d + b_d) -> [B,NH,T,L]
    att = softmax(causal(q @ k^T / sqrt(L)))     [B,NH,T,T]
    y   = att @ k -> [B,T,NH*L]
    out = y @ W_proj + b_proj                    [B,T,C]

Sharding: core = 4*b + g handles batch b, head group g (4 heads).  The latent
k is computed per-core (replicated), so there is no cross-device traffic.
Each core returns partial out = y_g @ W_proj[rows of g]; host sums the 4
group partials per batch and adds b_proj.

Device kernel layout notes:
  - All matmuls in bf16 (inputs cast on host), f32 PSUM accumulation.
  - Scores are computed transposed, ST[s,t] = k_s . q_t (per head), so
    exp(ST) is directly P^T, which is the stationary operand needed by the
    second matmul y[t,l] = sum_s P^T[s,t] * k_aug[s,l].  k_aug carries an
    extra ones column, making column L of y the softmax denominator Z[t].
  - Softmax skips max-subtraction: scores are O(+-5) here, exp is safe in f32.
  - Causal masking: off-diagonal [128s x 512t] blocks are fully valid or
    fully skipped; the 4 diagonal block flavours are masked by a
    precomputed 0/1 mask multiply after exp.
"""
import contextlib
import math
import sys

sys.path.insert(0, "/opt/trn_rl_repo")

import numpy as np
import ml_dtypes

import concourse.bass as bass
import concourse.mybir as mybir
import concourse.tile as tile
from concourse import bass_utils
from concourse.bass import ts
from concourse.vector_clock import ScopedClock

BF16 = ml_dtypes.bfloat16
F32 = mybir.dt.float32
F16 = mybir.dt.float16
BF = mybir.dt.bfloat16

B, T, C = 2, 2048, 2048
NH, L = 16, 128
HPC = 4          # heads per core
G = NH // HPC    # head groups (= cores per batch)
P = 128
CO = C // P      # c-tiles (contraction for q/k projections)
TT = T // P      # 128-row tiles of T
TB = T // 512    # 512-col blocks of T
SCALE = 1.0 / math.sqrt(L)

_PATCHED = False


def _patch_tile_drain():
    """This walrus build rejects instructions carrying more than one sync
    wait ("Too many sync wait commands", setupSyncWait).  Two patches:
    (1) a pass over the scheduled instruction lists that hoists all but one
    wait per instruction onto preceding same-engine NoOps, and (2) the
    TileContext tail drain emits one wait_ge instruction per semaphore
    instead of stacking waits on the Drain."""
    global _PATCHED
    if _PATCHED:
        return
    _PATCHED = True

    _orig_lower = tile.TileContext._lower_ordered_insts
    _ctr = [0]

    def _lower_ordered_insts(self, ordered):
        for bbname, insts in ordered.items():
            needs = any(
                i.sync_info is not None and len(i.sync_info.on_wait) > 1
                for i in insts
            )
            if not needs:
                continue
            newlist = []
            for inst in insts:
                si = inst.sync_info
                if si is not None and len(si.on_wait) > 1:
                    waits = list(si.on_wait)
                    for w in waits[:-1]:
                        nop = mybir.InstNoOp(
                            name=f"waitsplit_{_ctr[0]}", ins=[], outs=[],
                            engine=inst.engine,
                        )
                        _ctr[0] += 1
                        nop.sync_info = mybir.SyncInfo(on_wait=[w], on_update=[])
                        newlist.append(nop)
                    inst.sync_info = mybir.SyncInfo(
                        on_wait=[waits[-1]], on_update=list(si.on_update)
                    )
                newlist.append(inst)
            ordered[bbname] = newlist
        return _orig_lower(self, ordered)

    tile.TileContext._lower_ordered_insts = _lower_ordered_insts

    def _drain_and_barrier(self, tick_clock, wait_clock):
        nc = self.nc
        probe = mybir.InstNoOp(
            name="tail_wait_probe", ins=[], outs=[], engine=mybir.EngineType.SP
        )
        wait_clock.add_sem_waits(probe, ScopedClock({None: tick_clock.global_clock}))
        by_num = {h.num: h for h in wait_clock.sems.allocated().values()}
        if probe.sync_info is not None:
            for w in probe.sync_info.on_wait:
                nc.sync.wait_ge(by_num[w.id], w.wait_value)
        nc.sync.drain()
        nc.all_engine_barrier()
        assert self.sems is not None
        popped = nc._tile_sem_poison_stack.pop()
        assert popped is self._sem_poison
        nc.clear_and_free_semaphores(list(self.sems.allocated().values()))
        nc.all_engine_barrier()

    tile.TileContext._drain_and_barrier = _drain_and_barrier


_NC_CACHE = None


def build_nc():
    """Build the single-core Bass program (SPMD across the 8 cores)."""
    global _NC_CACHE
    if _NC_CACHE is not None:
        return _NC_CACHE
    _patch_tile_drain()

    nc = bass.Bass("TRN2", target_bir_lowering=False, debug=False)

    xT = nc.declare_dram_parameter("xT", [C, T], BF, isOutput=False)
    wd = nc.declare_dram_parameter("wd", [C, HPC * L], BF, isOutput=False)
    wlat = nc.declare_dram_parameter("wlat", [C, L], BF, isOutput=False)
    wproj = nc.declare_dram_parameter("wproj", [HPC * L, C], BF, isOutput=False)
    bd = nc.declare_dram_parameter("bd", [P, HPC], F32, isOutput=False)
    blat = nc.declare_dram_parameter("blat", [P, 1], F32, isOutput=False)
    mask = nc.declare_dram_parameter("mask", [P, 4 * 512], BF, isOutput=False)
    ident = nc.declare_dram_parameter("ident", [P, P], BF, isOutput=False)
    out = nc.declare_dram_parameter("out", [T, C], F16, isOutput=True)

    with tile.TileContext(nc) as tc:
        with (
            tc.tile_pool(name="const", bufs=1) as cp,
            tc.tile_pool(name="ptp", bufs=2) as ptp,
            tc.tile_pool(name="small", bufs=3) as sp,
            tc.tile_pool(name="outp", bufs=3) as op_,
        ):
            # ---- constant / persistent SBUF tiles ----
            xT_sb = cp.tile([P, CO, T], BF, tag="xT")
            wd_sb = cp.tile([P, CO, HPC * L], BF, tag="wd")
            wlat_sb = cp.tile([P, CO, L], BF, tag="wlat")
            wproj_sb = cp.tile([P, HPC, C], BF, tag="wproj")
            bd_sb = cp.tile([P, HPC], F32, tag="bd")
            blat_sb = cp.tile([P, 1], F32, tag="blat")
            mask_sb = cp.tile([P, 4 * 512], BF, tag="mask")
            ident_sb = cp.tile([P, P], BF, tag="ident")
            kT_sb = cp.tile([P, T], BF, tag="kT")            # [L, s]
            kaug_sb = cp.tile([P, TT, L + 8], BF, tag="kaug")  # [s, st, l|1]
            qT_sb = cp.tile([P, HPC, T], BF, tag="qT")       # [L, h, t]
            yT_sb = cp.tile([P, HPC, T], BF, tag="yT")       # [L, h, t]

            # Load order matters for the PE ramp: each ramp step co needs
            # wlat[co], wd[co] and xT[co]; xT streams one c-tile per DMA so
            # the ramp is never starved by one large transfer.  mask/wproj
            # only gate later phases.
            nc.scalar.dma_start(
                wlat_sb[:, 0:4, :], wlat[0 : 4 * P, :].rearrange("(o p) l -> p o l", p=P)
            )
            nc.sync.dma_start(
                xT_sb[:, 0:1, :], xT[0:P, :].rearrange("(o p) t -> p o t", p=P)
            )
            nc.scalar.dma_start(
                wd_sb[:, 0:4, :], wd[0 : 4 * P, :].rearrange("(o p) l -> p o l", p=P)
            )
            nc.scalar.dma_start(blat_sb[:], blat[:])
            nc.scalar.dma_start(bd_sb[:], bd[:])
            nc.scalar.dma_start(
                wlat_sb[:, 4:CO, :],
                wlat[4 * P :, :].rearrange("(o p) l -> p o l", p=P),
            )
            nc.scalar.dma_start(
                wd_sb[:, 4:CO, :], wd[4 * P :, :].rearrange("(o p) l -> p o l", p=P)
            )
            nc.scalar.dma_start(ident_sb[:], ident[:])
            xq = [(i, 1) for i in range(1, CO)]
            for off, n in xq:
                nc.sync.dma_start(
                    xT_sb[:, off : off + n, :],
                    xT[off * P : (off + n) * P, :].rearrange("(o p) t -> p o t", p=P),
                )
            nc.scalar.dma_start(mask_sb[:], mask[:])
            for q in range(2):
                nc.scalar.dma_start(
                    wproj_sb[:, 2 * q : 2 * q + 2, :],
                    wproj[ts(q, 2 * P), :].rearrange("(o p) c -> p o c", p=P),
                )

            # ---- phase 1 (ramp): kT and qT[h0] accumulate per c-tile as the
            # xT DMA batches land, keeping PE fed during the 8 MB load.
            # 8 concurrent PSUM accumulation groups in a scoped pool whose
            # banks are released to the later pools.
            with tc.tile_pool(name="ps_ramp", bufs=1, space="PSUM") as ramp:
                kps = [ramp.tile([P, 512], F32, tag=f"kps{tb}", name=f"kps{tb}") for tb in range(TB)]
                qps = [ramp.tile([P, 512], F32, tag=f"qps{tb}", name=f"qps{tb}") for tb in range(TB)]
                for co in range(CO):
                    for tb in range(TB):
                        nc.tensor.matmul(
                            kps[tb][:], wlat_sb[:, co, :], xT_sb[:, co, ts(tb, 512)],
                            start=(co == 0), stop=(co == CO - 1),
                        )
                    for tb in range(TB):
                        nc.tensor.matmul(
                            qps[tb][:], wd_sb[:, co, ts(0, P)], xT_sb[:, co, ts(tb, 512)],
                            start=(co == 0), stop=(co == CO - 1),
                        )
                for tb in range(TB):
                    nc.vector.tensor_scalar_add(
                        kT_sb[:, ts(tb, 512)], kps[tb][:], blat_sb[:, 0:1]
                    )
                    nc.vector.tensor_scalar_add(
                        qT_sb[:, 0, ts(tb, 512)], qps[tb][:], bd_sb[:, 0:1]
                    )

            main_pools = contextlib.ExitStack()
            # PSUM budget (8 banks): psST 2x[128,1024]=4, mm512 2, psy 1, pstr 1
            psST = main_pools.enter_context(
                tc.tile_pool(name="psST", bufs=2, space="PSUM")
            )
            ps512 = main_pools.enter_context(
                tc.tile_pool(name="ps512", bufs=2, space="PSUM")
            )
            psy = main_pools.enter_context(
                tc.tile_pool(name="psy", bufs=1, space="PSUM")
            )
            pstr = main_pools.enter_context(
                tc.tile_pool(name="pstr", bufs=1, space="PSUM")
            )

            # k_aug[s, l|1]: transpose kT tiles via identity matmul, ones col
            nc.vector.memset(kaug_sb[:, :, L : L + 1], 1.0)
            for st in range(TT):
                pt = pstr.tile([P, P], F32, tag="tr")
                nc.tensor.matmul(pt[:], kT_sb[:, ts(st, P)], ident_sb[:], start=True, stop=True)
                nc.vector.tensor_copy(out=kaug_sb[:, st, 0:L], in_=pt[:])

            def qT_phase(h):
                for tb in range(TB):
                    ps = ps512.tile([P, 512], F32, tag="mm512", name=f"q{h}{tb}")
                    for co in range(CO):
                        nc.tensor.matmul(
                            ps[:], wd_sb[:, co, ts(h, P)], xT_sb[:, co, ts(tb, 512)],
                            start=(co == 0), stop=(co == CO - 1),
                        )
                    nc.vector.tensor_scalar_add(
                        qT_sb[:, h, ts(tb, 512)], ps[:], bd_sb[:, h : h + 1]
                    )

            def att_block(J, h):
                """Attention for head h, query block J (512 t's)."""
                ns = 4 * J + 4  # s-tiles covering s <= max t of this block
                ptile = ptp.tile([P, TT, 512], BF, tag="pt", name=f"pt{J}{h}")
                # scores transposed, two s-tiles per PSUM tile.  Off-diagonal
                # pairs share one full-width exp; diagonal s-tiles (st = 4J+r)
                # only have valid t in [128r, 512) -> trim the matmul, exp and
                # mask to that range (the skipped PT region is never read by
                # the y matmuls, which use s-tile st only for t-tiles >= r).
                for sp_i in range(ns // 2):
                    ps = psST.tile([P, 2, 512], F32, tag="st", name=f"st{J}{h}{sp_i}")
                    st0 = 2 * sp_i
                    diag = st0 >= 4 * J
                    if not diag:
                        for k_ in range(2):
                            st = st0 + k_
                            nc.tensor.matmul(
                                ps[:, k_, :], kT_sb[:, ts(st, P)],
                                qT_sb[:, h, ts(J, 512)],
                                start=True, stop=True,
                            )
                        nc.scalar.activation(
                            ptile[:, st0 : st0 + 2, :], ps[:],
                            mybir.ActivationFunctionType.Exp,
                        )
                    else:
                        for k_ in range(2):
                            st = st0 + k_
                            r = st - 4 * J
                            nc.tensor.matmul(
                                ps[:, k_, P * r : 512], kT_sb[:, ts(st, P)],
                                qT_sb[:, h, 512 * J + P * r : 512 * (J + 1)],
                                start=True, stop=True,
                            )
                            nc.scalar.activation(
                                ptile[:, st, P * r : 512], ps[:, k_, P * r : 512],
                                mybir.ActivationFunctionType.Exp,
                            )
                            nc.vector.tensor_tensor(
                                ptile[:, st, P * r : 512],
                                ptile[:, st, P * r : 512],
                                mask_sb[:, 512 * r + P * r : 512 * (r + 1)],
                                mybir.AluOpType.mult,
                            )
                for tt in range(4):
                    tq = 4 * J + tt
                    nsy = tq + 1
                    py = psy.tile([P, 132], F32, tag="yaug", name=f"y{J}{h}{tt}")
                    for st in range(nsy):
                        nc.tensor.matmul(
                            py[:, 0 : L + 1],
                            ptile[:, st, ts(tt, P)],
                            kaug_sb[:, st, 0 : L + 1],
                            start=(st == 0), stop=(st == nsy - 1),
                        )
                    zr = sp.tile([P, 1], F32, tag="zr", name=f"zr{J}{h}{tt}")
                    nc.vector.reciprocal(zr[:], py[:, L : L + 1])
                    ynorm = sp.tile([P, P], BF, tag="ynorm", name=f"yn{J}{h}{tt}")
                    nc.vector.tensor_scalar_mul(ynorm[:], py[:, 0:L], zr[:])
                    ptr = pstr.tile([P, P], F32, tag="tr", name=f"ytr{J}{h}{tt}")
                    nc.tensor.matmul(ptr[:], ynorm[:], ident_sb[:], start=True, stop=True)
                    nc.vector.tensor_copy(out=yT_sb[:, h, ts(tq, P)], in_=ptr[:])

            def proj_rows(J):
                """out rows for t-tiles of block J (needs all heads' yT)."""
                for tq in range(4 * J, 4 * J + 4):
                    osb = op_.tile([P, C], F16, tag="osb", name=f"osb{tq}")
                    for cb in range(4):
                        ps = ps512.tile([P, 512], F32, tag="mm512", name=f"o{tq}{cb}")
                        for h in range(HPC):
                            nc.tensor.matmul(
                                ps[:], yT_sb[:, h, ts(tq, P)],
                                wproj_sb[:, h, ts(cb, 512)],
                                start=(h == 0), stop=(h == HPC - 1),
                            )
                        # all copies on DVE: ACT would pay a ~1.3us table swap
                        # whenever Copy interleaves with attention's Exp
                        nc.vector.tensor_copy(out=osb[:, ts(cb, 512)], in_=ps[:])
                        if tq >= TT - 2:
                            # last rows: drain each block immediately
                            nc.sync.dma_start(
                                out[ts(tq, P), ts(cb, 512)], osb[:, ts(cb, 512)]
                            )
                    if tq < TT - 2:
                        eng = nc.sync if tq % 2 == 0 else nc.scalar
                        eng.dma_start(out[ts(tq, P), :], osb[:])

            # ---- phases 1b + 2 interleaved: attention for (J=0, h) starts as
            # soon as qT[h] exists; qT[h+1] computes under the exp of block h.
            # Projection runs as a tail phase (interleaving it with attention
            # costs ~18us of PSUM-slot contention per the cost model).
            att_block(0, 0)
            for h in range(1, HPC):
                qT_phase(h)
                att_block(0, h)
            for J in range(1, TB):
                for h in range(HPC):
                    att_block(J, h)
            for J in range(TB):
                proj_rows(J)

            main_pools.close()

    _NC_CACHE = nc
    return nc


def make_in_maps(x, W_lat, b_lat, W_d, b_d, W_proj, b_proj):
    """Shard + preprocess full inputs into the 8 per-core input maps."""
    x = np.asarray(x, dtype=np.float32)
    W_lat = np.asarray(W_lat, dtype=np.float32)
    b_lat = np.asarray(b_lat, dtype=np.float32)
    W_d = np.asarray(W_d, dtype=np.float32)
    b_d = np.asarray(b_d, dtype=np.float32)
    W_proj = np.asarray(W_proj, dtype=np.float32)

    # causal mask for transposed-score diagonal blocks:
    # mask[p, r*512 + f] = 1 if f >= 128*r + p else 0
    pp = np.arange(P)[:, None]
    ff = np.arange(512)[None, :]
    mask = np.concatenate(
        [(ff >= P * r + pp) for r in range(4)], axis=1
    ).astype(BF16)
    ident = np.eye(P, dtype=BF16)
    wlat_b = W_lat.astype(BF16)
    blat_b = b_lat.reshape(P, 1).astype(np.float32)

    xT_b = [np.ascontiguousarray(x[b].T).astype(BF16) for b in range(B)]

    in_maps = []
    for core in range(8):
        b, g = core // G, core % G
        sl = slice(512 * g, 512 * (g + 1))
        in_maps.append({
            "xT": xT_b[b],
            "wd": (W_d[:, sl] * SCALE).astype(BF16),
            "wlat": wlat_b,
            "wproj": W_proj[sl, :].astype(BF16),
            "bd": np.ascontiguousarray(
                (b_d[sl] * SCALE).astype(np.float32).reshape(HPC, P).T
            ),
            "blat": blat_b,
            "mask": mask,
            "ident": ident,
        })
    return in_maps


def combine_outputs(per_core_out, b_proj):
    """Sum head-group partials per batch, add b_proj."""
    b_proj = np.asarray(b_proj, dtype=np.float32)
    outs = []
    for b in range(B):
        acc = per_core_out[G * b].astype(np.float32).copy()
        for g in range(1, G):
            acc += per_core_out[G * b + g]
        acc += b_proj[None, :]
        outs.append(acc)
    return np.stack(outs, axis=0)


def kernel(x, W_lat, b_lat, W_d, b_d, W_proj, b_proj):
    nc = build_nc()
    in_maps = make_in_maps(x, W_lat, b_lat, W_d, b_d, W_proj, b_proj)
    res = bass_utils.run_bass_kernel_spmd(nc, in_maps, core_ids=list(range(8)))
    per_core = [res.results[i]["out"] for i in range(8)]
    return combine_outputs(per_core, b_proj)
